# revision 13
# baseline (speedup 1.0000x reference)
"""FAPE loss Trainium2 kernel.

Math: for frames f (built from coord triples) and points n,
  d2[f,n] = ||Rp(p_n - po_f)||^2 + ||Rt(t_n - to_f)||^2 - 2 (p_n-po_f)^T M (t_n-to_f)
with M = Rp^T Rt.  Expanding, d2[f,n] = X[n] . Y[f] with 17 features:
  X = [A_n, 1, p (3), t (3), W (9)]   A_n = ||p_n||^2 + ||t_n||^2, W = outer(p_n, t_n)
  Y = [mask, B_f - 2c_f, 2(u-po), 2(v-to), -2M]  u = M to, v = M^T po,
      c_f = po.u, B_f = ||po||^2 + ||to||^2
Loss = mean(min(sqrt(d2 + eps), 10)) / 10 = mean(sqrt(max(min(d2,100),0) + eps))/10.

Sharding: frames split across 8 cores (512/core, 2 zero-masked pads on the
last); points replicated.  Each core computes d2 as K=17 matmuls
(lhsT = X^T point-group tiles, rhs = Y^T replicated at partition bases
0/32/64), clamps on DVE, sqrt+row-accumulates on ACT, and reduces to one
partial scalar.  Host sums the 8 partials.
"""
import sys

for _p in ("/opt/trn_rl_repo", "/root/.axon_site/_ro/trn_rl_repo"):
    if _p not in sys.path:
        sys.path.append(_p)

import numpy as np
from concourse import bass, bacc, mybir, tile, masks
from concourse.bass_utils import run_bass_kernel_spmd

F32 = mybir.dt.float32
AF = mybir.ActivationFunctionType
OP = mybir.AluOpType

N = 4096          # points
F = N - 2         # frames (4094)
NCORES = 8
FPC = 512         # frames per core (padded; last core has 510 real + 2 pad)
NGRP = 32         # point-groups of 128 (point n = 32p + m, group = m)
FCHUNK = FPC // 128  # 4 frame-chunks per core
CLAMP2 = 100.0
EPS = 1e-8
UNIT = 10.0
KF = 17           # contraction features
KPAD = 32         # feature stride
NBLK = 11         # X transpose windows of 96 cols (3 groups each)
XCOLS = 33 * KPAD  # Xall padded to 11 windows * 96 = 1056


def _frame_basis(nc, pool, fc, t0):
    """Gram-Schmidt frame basis for one tensor (t0=0 pred, 1 true).

    fc[t0] = [ca, cb, cc] coord tiles [128, 12] = (chunk4, comp3).
    Returns (e1n, e2n, e3, origin_view).
    """
    V = lambda ap: ap.rearrange("p (c j) -> p c j", j=3)
    ca, cb, cc = (t[:] for t in fc[t0])

    def normalize(vec, tagn):
        sq = pool.tile([128, 12], F32, tag=f"nsq{tagn}")
        nc.vector.tensor_mul(sq[:], vec[:], vec[:])
        dots = pool.tile([128, 4], F32, tag=f"ndt{tagn}")
        nc.vector.reduce_sum(dots[:], V(sq[:]), axis=mybir.AxisListType.X)
        nrm = pool.tile([128, 4], F32, tag=f"nnr{tagn}")
        nc.scalar.activation(nrm[:], dots[:], AF.Sqrt)
        nc.vector.tensor_scalar_add(nrm[:], nrm[:], EPS)
        rinv = pool.tile([128, 4], F32, tag=f"nri{tagn}")
        nc.vector.reciprocal(rinv[:], nrm[:])
        out = pool.tile([128, 12], F32, tag=f"nou{tagn}")
        rb = rinv[:][:, :, None].broadcast_to([128, 4, 3])
        nc.vector.tensor_mul(V(out[:]), V(vec[:]), rb)
        return out

    e1 = pool.tile([128, 12], F32, tag=f"e1_{t0}")
    nc.vector.tensor_sub(e1[:], cc, cb)
    e1n = normalize(e1, f"a{t0}")

    e2 = pool.tile([128, 12], F32, tag=f"e2_{t0}")
    nc.vector.tensor_sub(e2[:], ca, cb)
    pd = pool.tile([128, 12], F32, tag=f"pd_{t0}")
    nc.vector.tensor_mul(pd[:], e2[:], e1n[:])
    d12 = pool.tile([128, 4], F32, tag=f"d12_{t0}")
    nc.vector.reduce_sum(d12[:], V(pd[:]), axis=mybir.AxisListType.X)
    proj = pool.tile([128, 12], F32, tag=f"pj_{t0}")
    db = d12[:][:, :, None].broadcast_to([128, 4, 3])
    nc.vector.tensor_mul(V(proj[:]), V(e1n[:]), db)
    nc.vector.tensor_sub(e2[:], e2[:], proj[:])
    e2n = normalize(e2, f"b{t0}")

    # e3 = e1n x e2n, component-wise ([128,4] slices)
    e3 = pool.tile([128, 12], F32, tag=f"e3_{t0}")
    ta = pool.tile([128, 4], F32, tag=f"cx_{t0}")
    for j in range(3):
        j1, j2 = (j + 1) % 3, (j + 2) % 3
        a1 = V(e1n[:])[:, :, j1]
        a2 = V(e1n[:])[:, :, j2]
        b1 = V(e2n[:])[:, :, j1]
        b2 = V(e2n[:])[:, :, j2]
        nc.vector.tensor_mul(ta[:], a2, b1)
        ej = V(e3[:])[:, :, j]
        nc.vector.tensor_mul(ej, a1, b2)
        nc.vector.tensor_sub(ej, ej, ta[:])
    return e1n, e2n, e3, cb


def build_nc():
    nc = bacc.Bacc(None)

    xp_d = nc.dram_tensor("xp", [N, 3], F32, kind="ExternalInput")
    xt_d = nc.dram_tensor("xt", [N, 3], F32, kind="ExternalInput")
    fp_d = nc.dram_tensor("fp", [FPC + 2, 3], F32, kind="ExternalInput")
    ft_d = nc.dram_tensor("ft", [FPC + 2, 3], F32, kind="ExternalInput")
    vm_d = nc.dram_tensor("vm", [FPC], F32, kind="ExternalInput")
    out_d = nc.dram_tensor("out", [1, 1], F32, kind="ExternalOutput")

    with tile.TileContext(nc) as tc:
        with (
            tc.tile_pool(name="const", bufs=1) as constp,
            tc.tile_pool(name="inp", bufs=1) as inp,
            tc.tile_pool(name="xf", bufs=1) as xf,
            tc.tile_pool(name="xtb", bufs=NBLK) as xtb,
            tc.tile_pool(name="yprep", bufs=1) as yp,
            tc.tile_pool(name="psT", bufs=2, space="PSUM") as psT,
            tc.tile_pool(name="psD", bufs=5, space="PSUM") as psD,
            tc.tile_pool(name="psF", bufs=1, space="PSUM") as psF,
            tc.tile_pool(name="post", bufs=3) as post,
            tc.tile_pool(name="accp", bufs=1) as accp,
        ):
            # ---- constants
            ident = constp.tile([128, 128], F32)
            masks.make_identity(nc, ident[:])
            epst = constp.tile([128, 1], F32)
            nc.vector.memset(epst[:], EPS)
            ones = constp.tile([128, 1], F32)
            nc.vector.memset(ones[:], 1.0)

            # ---- input DMAs
            praw = inp.tile([128, 96], F32)  # point n = 32p + m -> [p, m*3+j]
            nc.sync.dma_start(praw[:], xp_d[:].rearrange("(p m) j -> p (m j)", p=128))
            traw = inp.tile([128, 96], F32)
            nc.sync.dma_start(traw[:], xt_d[:].rearrange("(p m) j -> p (m j)", p=128))

            # frame coords: 6 tiles [128, 12] = (chunk4, comp3), one DMA each
            fc = [[], []]
            for t0, fd in ((0, fp_d), (1, ft_d)):
                for s in range(3):
                    t = inp.tile([128, 12], F32, tag=f"fc{t0}{s}")
                    nc.sync.dma_start(
                        t[:].rearrange("p (c j) -> p c j", j=3),
                        fd[s: s + FPC].rearrange("(c p) j -> p c j", p=128),
                    )
                    fc[t0].append(t)
            vm_sb = inp.tile([128, 4], F32)
            nc.sync.dma_start(vm_sb[:], vm_d[:].rearrange("(c p) -> p c", p=128))

            # ---- X features: Xall[p, g*32 + k], g = 0..31 point-groups
            xall = xf.tile([128, XCOLS], F32)
            nc.vector.memset(xall[:], 0.0)
            xg = xall[:].rearrange("p (g k) -> p g k", k=KPAD)

            def xw(k):  # [128, 32] view of feature k (stride KPAD)
                return xg[:, 0:NGRP, k]

            def pcomp(raw, j):  # [128, 32] view of comp j (stride 3)
                return raw[:].rearrange("p (m j) -> p m j", j=3)[:, :, j]

            # A = sum of squares of all 6 coords
            scratch = xf.tile([128, 32], F32)
            nc.vector.tensor_mul(xw(0), pcomp(praw, 0), pcomp(praw, 0))
            for raw, j in ((praw, 1), (praw, 2), (traw, 0), (traw, 1), (traw, 2)):
                nc.vector.tensor_mul(scratch[:], pcomp(raw, j), pcomp(raw, j))
                nc.vector.tensor_add(xw(0), xw(0), scratch[:])
            nc.vector.memset(xw(1), 1.0)
            for j in range(3):
                nc.vector.tensor_copy(xw(2 + j), pcomp(praw, j))
                nc.vector.tensor_copy(xw(5 + j), pcomp(traw, j))
            for c in range(3):
                for d in range(3):
                    nc.vector.tensor_mul(xw(8 + 3 * c + d), pcomp(praw, c), pcomp(traw, d))

            # ---- transpose X in 96-col windows (3 groups per window so all
            # lhsT slices start at partition 0/32/64; 96 is illegal for PE)
            xts = []
            for b in range(NBLK):
                ps = psT.tile([96, 128], F32, tag="ps_tp")
                nc.tensor.transpose(ps[:], xall[:, b * 96: b * 96 + 96], ident[:])
                xt_t = xtb.tile([96, 128], F32, tag="xt_t")
                nc.vector.tensor_copy(xt_t[:], ps[:])
                xts.append(xt_t)

            # ---- Y features (frames on partitions, 4 chunks batched on free)
            e1p, e2p, e3p, po = _frame_basis(nc, yp, fc, 0)
            e1t, e2t, e3t, to = _frame_basis(nc, yp, fc, 1)

            V3 = lambda ap: ap.rearrange("p (c j) -> p c j", j=3)
            V33 = lambda ap: ap.rearrange("p (c i j) -> p c i j", i=3, j=3)

            # M[c,d] = sum_r ep_r[c] et_r[d]  -> [128, 36] = (chunk4, c3, d3)
            mw = yp.tile([128, 36], F32)
            mt = yp.tile([128, 36], F32)
            for r, (ep, et) in enumerate(((e1p, e1t), (e2p, e2t), (e3p, e3t))):
                epb = V3(ep[:])[:, :, :, None].broadcast_to([128, 4, 3, 3])
                etb = V3(et[:])[:, :, None, :].broadcast_to([128, 4, 3, 3])
                if r == 0:
                    nc.vector.tensor_mul(V33(mw[:]), epb, etb)
                else:
                    nc.vector.tensor_mul(V33(mt[:]), epb, etb)
                    nc.vector.tensor_add(mw[:], mw[:], mt[:])

            # u = M to (reduce over d), v = M^T po (reduce over c)
            prod = yp.tile([128, 36], F32)
            tob = V3(to)[:, :, None, :].broadcast_to([128, 4, 3, 3])
            nc.vector.tensor_mul(V33(prod[:]), V33(mw[:]), tob)
            u = yp.tile([128, 12], F32)
            nc.vector.reduce_sum(V3(u[:]), V33(prod[:]), axis=mybir.AxisListType.X)

            mw_t = V33(mw[:]).transpose([0, 1, 3, 2])  # (chunk, d, c)
            pob = V3(po)[:, :, None, :].broadcast_to([128, 4, 3, 3])
            nc.vector.tensor_mul(V33(prod[:]), mw_t, pob)
            v = yp.tile([128, 12], F32)
            nc.vector.reduce_sum(V3(v[:]), V33(prod[:]), axis=mybir.AxisListType.X)

            # c_f = po . u ; B = ||po||^2 + ||to||^2
            sm = yp.tile([128, 12], F32)
            nc.vector.tensor_mul(sm[:], po, u[:])
            cf = yp.tile([128, 4], F32)
            nc.vector.reduce_sum(cf[:], V3(sm[:]), axis=mybir.AxisListType.X)
            bsum = yp.tile([128, 4], F32)
            nc.vector.tensor_mul(sm[:], po, po)
            nc.vector.reduce_sum(bsum[:], V3(sm[:]), axis=mybir.AxisListType.X)
            nc.vector.tensor_mul(sm[:], to, to)
            b2 = yp.tile([128, 4], F32)
            nc.vector.reduce_sum(b2[:], V3(sm[:]), axis=mybir.AxisListType.X)
            nc.vector.tensor_add(bsum[:], bsum[:], b2[:])

            # ---- assemble Yassem [128, 128] = (chunk4, feat32)
            yassem = yp.tile([128, FCHUNK * KPAD], F32)
            nc.vector.memset(yassem[:], 0.0)
            yv = yassem[:].rearrange("p (c k) -> p c k", k=KPAD)

            nc.vector.memset(yv[:, :, 0], 1.0)
            # k1 = B - 2 c_f
            nc.vector.scalar_tensor_tensor(
                yv[:, :, 1], cf[:], -2.0, bsum[:], OP.mult, OP.add
            )
            # k2..4 = 2(u - po) ; k5..7 = 2(v - to)
            diff = yp.tile([128, 12], F32)
            nc.vector.tensor_sub(diff[:], u[:], po)
            nc.vector.tensor_scalar_mul(
                yv[:, :, 2:5], diff[:].rearrange("p (c j) -> p c j", j=3), 2.0
            )
            nc.vector.tensor_sub(diff[:], v[:], to)
            nc.vector.tensor_scalar_mul(
                yv[:, :, 5:8], diff[:].rearrange("p (c j) -> p c j", j=3), 2.0
            )
            # k8..16 = -2 M
            nc.vector.tensor_scalar_mul(
                yv[:, :, 8:17], V33(mw[:]).rearrange("p c i j -> p c (i j)"), -2.0
            )
            # mask out pad frames (vmask = 0): multiply every feature
            vb = vm_sb[:][:, :, None].broadcast_to([128, 4, KPAD])
            nc.vector.tensor_mul(yv[:, :, 0:KPAD], yv[:, :, 0:KPAD], vb)

            # ---- replicate features 4x within each chunk's 128-col window,
            # transpose per chunk, evac into rhs4 [128, 512]: rows r*32+k all
            # hold Y^T so lhsT slices at bases 0/32/64 find a matching rhs
            yrep = yp.tile([128, FCHUNK * 128], F32)
            yrv = yrep[:].rearrange("p (c r k) -> p c r k", r=4, k=KPAD)
            ysrc = yv[:, :, None, :].broadcast_to([128, 4, 4, KPAD])
            nc.vector.tensor_copy(yrv, ysrc)

            rhs4 = yp.tile([128, FPC], F32)
            for c in range(FCHUNK):
                psy = psT.tile([128, 128], F32, tag="ps_tp")
                nc.tensor.transpose(psy[:], yrep[:, c * 128: (c + 1) * 128], ident[:])
                nc.vector.tensor_copy(rhs4[:, c * 128: (c + 1) * 128], psy[:])

            # ---- main: 32 x (matmul K=17 -> clamp -> sqrt+accum)
            acc = accp.tile([128, NGRP], F32)
            for g in range(NGRP):
                b, s = divmod(g, 3)
                lhsT = xts[b][s * KPAD: s * KPAD + KF, :]
                rhs_r = rhs4[s * KPAD: s * KPAD + KF, :]
                ps = psD.tile([128, FPC], F32, tag="d2")
                nc.tensor.matmul(ps[:], lhsT, rhs_r, start=True, stop=True)
                smin = post.tile([128, FPC], F32, tag="smin")
                nc.vector.tensor_scalar(smin[:], ps[:], CLAMP2, 0.0, OP.min, OP.max)
                ssq = post.tile([128, FPC], F32, tag="ssq")
                nc.scalar.activation(
                    ssq[:], smin[:], AF.Sqrt, bias=epst[:],
                    accum_out=acc[:, g: g + 1],
                )

            # ---- tail: acc [128,32] -> [128,1] -> [1,1]
            accs = accp.tile([128, 1], F32)
            nc.vector.reduce_sum(accs[:], acc[:], axis=mybir.AxisListType.X)
            psf = psF.tile([1, 1], F32)
            nc.tensor.matmul(psf[:], accs[:], ones[:], start=True, stop=True)
            outsb = accp.tile([1, 1], F32)
            nc.vector.tensor_copy(outsb[:], psf[:])
            nc.sync.dma_start(out_d[:], outsb[:])

    nc.finalize()
    return nc


_NC_CACHE = None


def _get_nc():
    global _NC_CACHE
    if _NC_CACHE is None:
        _NC_CACHE = build_nc()
    return _NC_CACHE


def make_in_maps(pred_coords, true_coords):
    pred = np.ascontiguousarray(pred_coords, dtype=np.float32)
    true = np.ascontiguousarray(true_coords, dtype=np.float32)
    in_maps = []
    for i in range(NCORES):
        f0 = i * FPC
        fp = np.zeros((FPC + 2, 3), np.float32)
        ft = np.zeros((FPC + 2, 3), np.float32)
        hi = min(f0 + FPC + 2, N)
        fp[: hi - f0] = pred[f0:hi]
        ft[: hi - f0] = true[f0:hi]
        vm = np.ones(FPC, np.float32)
        nvalid = max(0, min(FPC, F - f0))
        vm[nvalid:] = 0.0
        in_maps.append({"xp": pred, "xt": true, "fp": fp, "ft": ft, "vm": vm})
    return in_maps


def kernel(pred_coords, true_coords):
    nc = _get_nc()
    in_maps = make_in_maps(pred_coords, true_coords)
    res = run_bass_kernel_spmd(nc, in_maps, list(range(NCORES)))
    total = sum(float(r["out"][0, 0]) for r in res.results)
    return np.float32(total / (F * N) / UNIT)


# revision 21
# speedup vs baseline: 1.1651x; 1.1651x over previous
"""FAPE loss Trainium2 kernel.

Math: for frames f (built from coord triples) and points n,
  d2[f,n] = ||Rp(p_n - po_f)||^2 + ||Rt(t_n - to_f)||^2 - 2 (p_n-po_f)^T M (t_n-to_f)
with M = Rp^T Rt.  Expanding, d2[f,n] = X[n] . Y[f] with 17 features:
  X = [A_n, 1, p (3), t (3), W (9)]   A_n = ||p_n||^2 + ||t_n||^2, W = outer(p_n, t_n)
  Y = [mask, B_f - 2c_f, 2(u-po), 2(v-to), -2M]  u = M to, v = M^T po,
      c_f = po.u, B_f = ||po||^2 + ||to||^2
Loss = mean(min(sqrt(d2 + eps), 10)) / 10 = mean(sqrt(max(min(d2,100),0) + eps))/10.

Sharding: frames split across 8 cores (512/core, 2 zero-masked pads on the
last); points replicated.  Each core computes d2 as K=17 float32r matmuls
(lhsT = X^T point-group tiles, rhs = Y^T replicated at partition bases
0/32/64), clamps on DVE, sqrt+row-accumulates on ACT, and reduces to one
partial scalar.  Host sums the 8 partials.
"""
import sys

for _p in ("/opt/trn_rl_repo", "/root/.axon_site/_ro/trn_rl_repo"):
    if _p not in sys.path:
        sys.path.append(_p)

import numpy as np
from concourse import bass, bacc, mybir, tile, masks
from concourse.bass_utils import run_bass_kernel_spmd

F32 = mybir.dt.float32
F32R = mybir.dt.float32r
AF = mybir.ActivationFunctionType
OP = mybir.AluOpType

N = 4096          # points
F = N - 2         # frames (4094)
NCORES = 8
FPC = 512         # frames per core (padded; last core has 510 real + 2 pad)
NGRP = 32         # point-groups of 128 (point n = 32p + m, group = m)
FCHUNK = FPC // 128  # 4 frame-chunks per core
CLAMP2 = 100.0
EPS = 1e-8
UNIT = 10.0
KF = 17           # contraction features
KPAD = 32         # feature stride
NBLK = 11         # X transpose windows of 96 cols (3 groups each)
XCOLS = 33 * KPAD  # Xall padded to 11 windows * 96 = 1056
USE_F32R = True   # single-pass fp32r matmul (4x faster than fp32)


MMDT = F32R if USE_F32R else F32


def _frame_basis(nc, pool, fc):
    """Gram-Schmidt frame bases for pred+true batched on [128, 24] tiles.

    fc = [ca, cb, cc] tiles [128, 24] = (tensor2, chunk4, comp3).
    Returns (e1n, e2n, e3, origin) tiles [128, 24].
    """
    V = lambda ap: ap.rearrange("p (t c j) -> p t c j", t=2, j=3)
    ca, cb, cc = (t[:] for t in fc)

    def normalize(vec, tagn):
        sq = pool.tile([128, 24], F32, tag=f"nsq{tagn}")
        nc.vector.tensor_mul(sq[:], vec[:], vec[:])
        dots = pool.tile([128, 8], F32, tag=f"ndt{tagn}")
        nc.vector.reduce_sum(dots[:], V(sq[:]), axis=mybir.AxisListType.X)
        nrm = pool.tile([128, 8], F32, tag=f"nnr{tagn}")
        nc.scalar.activation(nrm[:], dots[:], AF.Sqrt)
        nc.vector.tensor_scalar_add(nrm[:], nrm[:], EPS)
        rinv = pool.tile([128, 8], F32, tag=f"nri{tagn}")
        nc.vector.reciprocal(rinv[:], nrm[:])
        out = pool.tile([128, 24], F32, tag=f"nou{tagn}")
        rb = rinv[:].rearrange("p (t c) -> p t c", t=2)[:, :, :, None]
        nc.vector.tensor_mul(V(out[:]), V(vec[:]), rb.broadcast_to([128, 2, 4, 3]))
        return out

    e1 = pool.tile([128, 24], F32, tag="e1")
    nc.vector.tensor_sub(e1[:], cc, cb)
    e1n = normalize(e1, "a")

    e2 = pool.tile([128, 24], F32, tag="e2")
    nc.vector.tensor_sub(e2[:], ca, cb)
    pd = pool.tile([128, 24], F32, tag="pd")
    nc.vector.tensor_mul(pd[:], e2[:], e1n[:])
    d12 = pool.tile([128, 8], F32, tag="d12")
    nc.vector.reduce_sum(d12[:], V(pd[:]), axis=mybir.AxisListType.X)
    proj = pool.tile([128, 24], F32, tag="pj")
    db = d12[:].rearrange("p (t c) -> p t c", t=2)[:, :, :, None]
    nc.vector.tensor_mul(V(proj[:]), V(e1n[:]), db.broadcast_to([128, 2, 4, 3]))
    nc.vector.tensor_sub(e2[:], e2[:], proj[:])
    e2n = normalize(e2, "b")

    # e3 = e1n x e2n, component-wise ([128, 8] slices across both tensors)
    e3 = pool.tile([128, 24], F32, tag="e3")
    ta = pool.tile([128, 8], F32, tag="cx")
    for j in range(3):
        j1, j2 = (j + 1) % 3, (j + 2) % 3
        a1 = V(e1n[:])[:, :, :, j1]
        a2 = V(e1n[:])[:, :, :, j2]
        b1 = V(e2n[:])[:, :, :, j1]
        b2 = V(e2n[:])[:, :, :, j2]
        tav = ta[:].rearrange("p (t c) -> p t c", t=2)
        nc.vector.tensor_mul(tav, a2, b1)
        ej = V(e3[:])[:, :, :, j]
        nc.vector.tensor_mul(ej, a1, b2)
        nc.vector.tensor_sub(ej, ej, tav)
    return e1n, e2n, e3, cb


def build_nc():
    nc = bacc.Bacc(None)

    xp_d = nc.dram_tensor("xp", [N, 3], F32, kind="ExternalInput")
    xt_d = nc.dram_tensor("xt", [N, 3], F32, kind="ExternalInput")
    fp_d = nc.dram_tensor("fp", [FPC + 2, 3], F32, kind="ExternalInput")
    ft_d = nc.dram_tensor("ft", [FPC + 2, 3], F32, kind="ExternalInput")
    vm_d = nc.dram_tensor("vm", [FPC], F32, kind="ExternalInput")
    out_d = nc.dram_tensor("out", [1, 1], F32, kind="ExternalOutput")

    with tile.TileContext(nc) as tc:
        with (
            tc.tile_pool(name="const", bufs=1) as constp,
            tc.tile_pool(name="inp", bufs=1) as inp,
            tc.tile_pool(name="xf", bufs=1) as xf,
            tc.tile_pool(name="xtb", bufs=NBLK) as xtb,
            tc.tile_pool(name="yprep", bufs=1) as yp,
            tc.tile_pool(name="psT", bufs=2, space="PSUM") as psT,
            tc.tile_pool(name="psD", bufs=3, space="PSUM") as psD,
            tc.tile_pool(name="post", bufs=3) as post,
            tc.tile_pool(name="accp", bufs=1) as accp,
        ):
            # ---- constants
            ident = constp.tile([128, 128], F32)
            masks.make_identity(nc, ident[:])
            epst = constp.tile([128, 1], F32)
            nc.vector.memset(epst[:], EPS)
            ones = constp.tile([128, 1], F32)
            nc.vector.memset(ones[:], 1.0)

            # ---- input DMAs
            praw = inp.tile([128, 96], F32)  # point n = 32p + m -> [p, m*3+j]
            nc.sync.dma_start(praw[:], xp_d[:].rearrange("(p m) j -> p (m j)", p=128))
            traw = inp.tile([128, 96], F32)
            nc.sync.dma_start(traw[:], xt_d[:].rearrange("(p m) j -> p (m j)", p=128))

            # frame coords: 3 tiles [128, 24] = (tensor2, chunk4, comp3)
            fc = []
            for s in range(3):
                t = inp.tile([128, 24], F32, tag=f"fc{s}")
                for t0, fd in ((0, fp_d), (1, ft_d)):
                    nc.sync.dma_start(
                        t[:, t0 * 12: t0 * 12 + 12].rearrange("p (c j) -> p c j", j=3),
                        fd[s: s + FPC].rearrange("(c p) j -> p c j", p=128),
                    )
                fc.append(t)
            vm_sb = inp.tile([128, 4], F32)
            nc.sync.dma_start(vm_sb[:], vm_d[:].rearrange("(c p) -> p c", p=128))

            # ---- X features: Xall[p, g*32 + k], g = 0..31 point-groups
            xall = xf.tile([128, XCOLS], F32)
            nc.vector.memset(xall[:], 0.0)
            xg = xall[:].rearrange("p (m k) -> p m k", k=KPAD)[:, 0:NGRP, :]

            pv = praw[:].rearrange("p (m j) -> p m j", j=3)
            tv = traw[:].rearrange("p (m j) -> p m j", j=3)

            # A = |p|^2 + |t|^2: square both, group-reduce, add
            sqp = xf.tile([128, 96], F32)
            nc.vector.tensor_mul(sqp[:], praw[:], praw[:])
            nc.vector.reduce_sum(
                xg[:, :, 0], sqp[:].rearrange("p (m j) -> p m j", j=3),
                axis=mybir.AxisListType.X,
            )
            nc.vector.tensor_mul(sqp[:], traw[:], traw[:])
            scr = xf.tile([128, 32], F32)
            nc.vector.reduce_sum(scr[:], sqp[:].rearrange("p (m j) -> p m j", j=3), axis=mybir.AxisListType.X)
            nc.vector.tensor_add(xg[:, :, 0], xg[:, :, 0], scr[:])
            nc.vector.memset(xg[:, :, 1], 1.0)
            # p, t coords (single strided copies)
            nc.vector.tensor_copy(xg[:, :, 2:5], pv)
            nc.vector.tensor_copy(xg[:, :, 5:8], tv)
            # W = outer(p, t): one op via double broadcast
            wout = xg[:, :, 8:17].rearrange("p m (c d) -> p m c d", d=3)
            pb = pv[:, :, :, None].broadcast_to([128, NGRP, 3, 3])
            tb = tv[:, :, None, :].broadcast_to([128, NGRP, 3, 3])
            nc.vector.tensor_mul(wout, pb, tb)

            # ---- transpose X in 96-col windows (3 groups per window so all
            # lhsT slices start at partition 0/32/64; 96 is illegal for PE)
            xts = []
            for b in range(NBLK):
                ps = psT.tile([96, 128], F32, tag="ps_tp")
                nc.tensor.transpose(ps[:], xall[:, b * 96: b * 96 + 96], ident[:])
                xt_t = xtb.tile([96, 128], MMDT, tag="xt_t")
                nc.vector.tensor_copy(xt_t[:], ps[:])
                xts.append(xt_t)

            # ---- Y features (frames on partitions, pred+true batched)
            e1n, e2n, e3, orig = _frame_basis(nc, yp, fc)
            po = orig[:, 0:12]
            to = orig[:, 12:24]

            V3 = lambda ap: ap.rearrange("p (c j) -> p c j", j=3)
            V33 = lambda ap: ap.rearrange("p (c i j) -> p c i j", i=3, j=3)

            # M[c,d] = sum_r ep_r[c] et_r[d]  -> [128, 36] = (chunk4, c3, d3)
            mw = yp.tile([128, 36], F32)
            mt = yp.tile([128, 36], F32)
            for r, e in enumerate((e1n, e2n, e3)):
                epb = V3(e[:, 0:12])[:, :, :, None].broadcast_to([128, 4, 3, 3])
                etb = V3(e[:, 12:24])[:, :, None, :].broadcast_to([128, 4, 3, 3])
                if r == 0:
                    nc.vector.tensor_mul(V33(mw[:]), epb, etb)
                else:
                    nc.vector.tensor_mul(V33(mt[:]), epb, etb)
                    nc.vector.tensor_add(mw[:], mw[:], mt[:])

            # u = M to (reduce over d), v = M^T po (reduce over c)
            prod = yp.tile([128, 36], F32)
            tob = V3(to)[:, :, None, :].broadcast_to([128, 4, 3, 3])
            nc.vector.tensor_mul(V33(prod[:]), V33(mw[:]), tob)
            u = yp.tile([128, 12], F32)
            nc.vector.reduce_sum(V3(u[:]), V33(prod[:]), axis=mybir.AxisListType.X)

            mw_t = V33(mw[:]).transpose([0, 1, 3, 2])  # (chunk, d, c)
            pob = V3(po)[:, :, None, :].broadcast_to([128, 4, 3, 3])
            nc.vector.tensor_mul(V33(prod[:]), mw_t, pob)
            v = yp.tile([128, 12], F32)
            nc.vector.reduce_sum(V3(v[:]), V33(prod[:]), axis=mybir.AxisListType.X)

            # c_f = po . u ; B = ||po||^2 + ||to||^2
            sm = yp.tile([128, 24], F32)
            nc.vector.tensor_mul(sm[:, 0:12], po, u[:])
            cf = yp.tile([128, 4], F32)
            nc.vector.reduce_sum(cf[:], V3(sm[:, 0:12]), axis=mybir.AxisListType.X)
            nc.vector.tensor_mul(sm[:], orig[:], orig[:])
            b8 = yp.tile([128, 8], F32)
            nc.vector.reduce_sum(
                b8[:].rearrange("p (t c) -> p t c", t=2),
                sm[:].rearrange("p (t c j) -> p t c j", t=2, j=3),
                axis=mybir.AxisListType.X,
            )
            bsum = yp.tile([128, 4], F32)
            nc.vector.tensor_add(bsum[:], b8[:, 0:4], b8[:, 4:8])

            # ---- assemble Yassem [128, 128] = (chunk4, feat32)
            yassem = yp.tile([128, FCHUNK * KPAD], F32)
            nc.vector.memset(yassem[:], 0.0)
            yv = yassem[:].rearrange("p (c k) -> p c k", k=KPAD)

            nc.vector.memset(yv[:, :, 0], 1.0)
            # k1 = B - 2 c_f
            nc.vector.scalar_tensor_tensor(
                yv[:, :, 1], cf[:], -2.0, bsum[:], OP.mult, OP.add
            )
            # k2..4 = 2(u - po) ; k5..7 = 2(v - to)
            diff = yp.tile([128, 12], F32)
            nc.vector.tensor_sub(diff[:], u[:], po)
            nc.vector.tensor_scalar_mul(yv[:, :, 2:5], V3(diff[:]), 2.0)
            nc.vector.tensor_sub(diff[:], v[:], to)
            nc.vector.tensor_scalar_mul(yv[:, :, 5:8], V3(diff[:]), 2.0)
            # k8..16 = -2 M
            nc.vector.tensor_scalar_mul(
                yv[:, :, 8:17], V33(mw[:]).rearrange("p c i j -> p c (i j)"), -2.0
            )
            # mask out pad frames (vmask = 0): multiply every feature
            vb = vm_sb[:][:, :, None].broadcast_to([128, 4, KPAD])
            nc.vector.tensor_mul(yv[:, :, 0:KPAD], yv[:, :, 0:KPAD], vb)

            # ---- replicate features 4x within each chunk's 128-col window,
            # transpose per chunk, evac into rhs4 [128, 512]: rows r*32+k all
            # hold Y^T so lhsT slices at bases 0/32/64 find a matching rhs
            yrep = yp.tile([128, FCHUNK * 128], F32)
            yrv = yrep[:].rearrange("p (c r k) -> p c r k", r=4, k=KPAD)
            ysrc = yv[:, :, None, :].broadcast_to([128, 4, 4, KPAD])
            nc.vector.tensor_copy(yrv, ysrc)

            rhs4 = yp.tile([128, FPC], MMDT)
            for c in range(FCHUNK):
                psy = psT.tile([128, 128], F32, tag="ps_tp")
                nc.tensor.transpose(psy[:], yrep[:, c * 128: (c + 1) * 128], ident[:])
                nc.vector.tensor_copy(rhs4[:, c * 128: (c + 1) * 128], psy[:])

            # ---- main: 16 x (2 matmuls K=17 -> clamp -> sqrt+accum) on
            # [128, 1024] double-bank PSUM tiles
            acc = accp.tile([128, NGRP // 2], F32)
            for i in range(NGRP // 2):
                ps = psD.tile([128, 2 * FPC], F32, tag="d2")
                for h in range(2):
                    g = 2 * i + h
                    b, s = divmod(g, 3)
                    lhsT = xts[b][s * KPAD: s * KPAD + KF, :]
                    rhs_r = rhs4[s * KPAD: s * KPAD + KF, :]
                    nc.tensor.matmul(
                        ps[:, h * FPC: (h + 1) * FPC],
                        lhsT, rhs_r,
                        start=True, stop=True,
                    )
                smin = post.tile([128, 2 * FPC], F32, tag="smin")
                nc.vector.tensor_scalar(smin[:], ps[:], CLAMP2, 0.0, OP.min, OP.max)
                ssq = post.tile([128, 2 * FPC], F32, tag="ssq")
                nc.scalar.activation(
                    ssq[:], smin[:], AF.Sqrt, bias=epst[:],
                    accum_out=acc[:, i: i + 1],
                )

            # ---- tail: acc [128,16] -> [128,1] -> [1,1]
            accs = accp.tile([128, 1], F32)
            nc.vector.reduce_sum(accs[:], acc[:], axis=mybir.AxisListType.X)
            psf = psT.tile([1, 1], F32, tag="ps_tp")
            nc.tensor.matmul(psf[:], accs[:], ones[:], start=True, stop=True)
            outsb = accp.tile([1, 1], F32)
            nc.vector.tensor_copy(outsb[:], psf[:])
            nc.sync.dma_start(out_d[:], outsb[:])

    nc.finalize()
    return nc


_NC_CACHE = None


def _get_nc():
    global _NC_CACHE
    if _NC_CACHE is None:
        _NC_CACHE = build_nc()
    return _NC_CACHE


def make_in_maps(pred_coords, true_coords):
    pred = np.ascontiguousarray(pred_coords, dtype=np.float32)
    true = np.ascontiguousarray(true_coords, dtype=np.float32)
    in_maps = []
    for i in range(NCORES):
        f0 = i * FPC
        fp = np.zeros((FPC + 2, 3), np.float32)
        ft = np.zeros((FPC + 2, 3), np.float32)
        hi = min(f0 + FPC + 2, N)
        fp[: hi - f0] = pred[f0:hi]
        ft[: hi - f0] = true[f0:hi]
        vm = np.ones(FPC, np.float32)
        nvalid = max(0, min(FPC, F - f0))
        vm[nvalid:] = 0.0
        in_maps.append({"xp": pred, "xt": true, "fp": fp, "ft": ft, "vm": vm})
    return in_maps


def kernel(pred_coords, true_coords):
    nc = _get_nc()
    in_maps = make_in_maps(pred_coords, true_coords)
    res = run_bass_kernel_spmd(nc, in_maps, list(range(NCORES)))
    total = sum(float(r["out"][0, 0]) for r in res.results)
    return np.float32(total / (F * N) / UNIT)


# revision 27
# speedup vs baseline: 1.2237x; 1.0503x over previous
"""FAPE loss Trainium2 kernel.

Math: for frames f (built from coord triples) and points n,
  d2[f,n] = ||Rp(p_n - po_f)||^2 + ||Rt(t_n - to_f)||^2 - 2 (p_n-po_f)^T M (t_n-to_f)
with M = Rp^T Rt.  Expanding, d2[f,n] = X[n] . Y[f] with 17 features:
  X = [A_n, 1, p (3), t (3), W (9)]   A_n = ||p_n||^2 + ||t_n||^2, W = outer(p_n, t_n)
  Y = [mask, B_f - 2c_f, 2(u-po), 2(v-to), -2M]  u = M to, v = M^T po,
      c_f = po.u, B_f = ||po||^2 + ||to||^2
Loss = mean(min(sqrt(d2 + eps), 10)) / 10 = mean(sqrt(max(min(d2,100),0) + eps))/10.

Sharding: frames split across 8 cores (512/core, 2 zero-masked pads on the
last); points replicated.  Each core computes d2 as K=17 float32r matmuls
(lhsT = X^T point-group tiles, rhs = Y^T replicated at partition bases
0/32/64), clamps on DVE, sqrt+row-accumulates on ACT, and reduces to one
partial scalar.  Host sums the 8 partials.
"""
import sys

for _p in ("/opt/trn_rl_repo", "/root/.axon_site/_ro/trn_rl_repo"):
    if _p not in sys.path:
        sys.path.append(_p)

import numpy as np
from concourse import bass, bacc, mybir, tile, masks
from concourse.bass_utils import run_bass_kernel_spmd

F32 = mybir.dt.float32
F32R = mybir.dt.float32r
BF16 = mybir.dt.bfloat16
AF = mybir.ActivationFunctionType
OP = mybir.AluOpType

N = 4096          # points
F = N - 2         # frames (4094)
NCORES = 8
FPC = 512         # frames per core (padded; last core has 510 real + 2 pad)
NGRP = 32         # point-groups of 128 (point n = 32p + m, group = m)
FCHUNK = FPC // 128  # 4 frame-chunks per core
CLAMP2 = 100.0
EPS = 1e-8
UNIT = 10.0
KF = 17           # contraction features
KPAD = 32         # feature stride
NBLK = 11         # X transpose windows of 96 cols (3 groups each)
XCOLS = 33 * KPAD  # Xall padded to 11 windows * 96 = 1056
USE_F32R = True   # single-pass fp32r matmul (4x faster than fp32)
DSQ_OFF = 0.25    # added to every d2 so f32r noise can't push it negative
                  # (sqrt(neg)=NaN); biases the loss by ~3e-5 relative


MMDT = F32R if USE_F32R else F32


def _frame_basis(nc, pool, fc):
    """Gram-Schmidt frame bases for pred+true batched on [128, 24] tiles.

    fc = [ca, cb, cc] tiles [128, 24] = (tensor2, chunk4, comp3).
    Returns (e1n, e2n, e3, origin) tiles [128, 24].
    """
    V = lambda ap: ap.rearrange("p (t c j) -> p t c j", t=2, j=3)
    ca, cb, cc = (t[:] for t in fc)

    def normalize(vec, tagn):
        sq = pool.tile([128, 24], F32, tag=f"nsq{tagn}")
        nc.vector.tensor_mul(sq[:], vec[:], vec[:])
        dots = pool.tile([128, 8], F32, tag=f"ndt{tagn}")
        nc.vector.reduce_sum(dots[:], V(sq[:]), axis=mybir.AxisListType.X)
        nrm = pool.tile([128, 8], F32, tag=f"nnr{tagn}")
        nc.scalar.activation(nrm[:], dots[:], AF.Sqrt)
        nc.vector.tensor_scalar_add(nrm[:], nrm[:], EPS)
        rinv = pool.tile([128, 8], F32, tag=f"nri{tagn}")
        nc.vector.reciprocal(rinv[:], nrm[:])
        out = pool.tile([128, 24], F32, tag=f"nou{tagn}")
        rb = rinv[:].rearrange("p (t c) -> p t c", t=2)[:, :, :, None]
        nc.vector.tensor_mul(V(out[:]), V(vec[:]), rb.broadcast_to([128, 2, 4, 3]))
        return out

    e1 = pool.tile([128, 24], F32, tag="e1")
    nc.vector.tensor_sub(e1[:], cc, cb)
    e1n = normalize(e1, "a")

    e2 = pool.tile([128, 24], F32, tag="e2")
    nc.vector.tensor_sub(e2[:], ca, cb)
    pd = pool.tile([128, 24], F32, tag="pd")
    nc.vector.tensor_mul(pd[:], e2[:], e1n[:])
    d12 = pool.tile([128, 8], F32, tag="d12")
    nc.vector.reduce_sum(d12[:], V(pd[:]), axis=mybir.AxisListType.X)
    proj = pool.tile([128, 24], F32, tag="pj")
    db = d12[:].rearrange("p (t c) -> p t c", t=2)[:, :, :, None]
    nc.vector.tensor_mul(V(proj[:]), V(e1n[:]), db.broadcast_to([128, 2, 4, 3]))
    nc.vector.tensor_sub(e2[:], e2[:], proj[:])
    e2n = normalize(e2, "b")

    # e3 = e1n x e2n, component-wise ([128, 8] slices across both tensors)
    e3 = pool.tile([128, 24], F32, tag="e3")
    ta = pool.tile([128, 8], F32, tag="cx")
    for j in range(3):
        j1, j2 = (j + 1) % 3, (j + 2) % 3
        a1 = V(e1n[:])[:, :, :, j1]
        a2 = V(e1n[:])[:, :, :, j2]
        b1 = V(e2n[:])[:, :, :, j1]
        b2 = V(e2n[:])[:, :, :, j2]
        tav = ta[:].rearrange("p (t c) -> p t c", t=2)
        nc.vector.tensor_mul(tav, a2, b1)
        ej = V(e3[:])[:, :, :, j]
        nc.vector.tensor_mul(ej, a1, b2)
        nc.vector.tensor_sub(ej, ej, tav)
    return e1n, e2n, e3, cb


def build_nc():
    nc = bacc.Bacc(None)

    xp_d = nc.dram_tensor("xp", [N, 3], F32, kind="ExternalInput")
    xt_d = nc.dram_tensor("xt", [N, 3], F32, kind="ExternalInput")
    fp_d = nc.dram_tensor("fp", [FPC + 2, 3], F32, kind="ExternalInput")
    ft_d = nc.dram_tensor("ft", [FPC + 2, 3], F32, kind="ExternalInput")
    vm_d = nc.dram_tensor("vm", [FPC], F32, kind="ExternalInput")
    out_d = nc.dram_tensor("out", [1, 1], F32, kind="ExternalOutput")

    with tile.TileContext(nc) as tc:
        with (
            tc.tile_pool(name="const", bufs=1) as constp,
            tc.tile_pool(name="inp", bufs=1) as inp,
            tc.tile_pool(name="xf", bufs=1) as xf,
            tc.tile_pool(name="xtb", bufs=NBLK) as xtb,
            tc.tile_pool(name="yprep", bufs=1) as yp,
            tc.tile_pool(name="psT", bufs=2, space="PSUM") as psT,
            tc.tile_pool(name="psD", bufs=3, space="PSUM") as psD,
            tc.tile_pool(name="post", bufs=3) as post,
            tc.tile_pool(name="accp", bufs=1) as accp,
        ):
            # ---- constants
            ident = constp.tile([128, 128], F32)
            masks.make_identity(nc, ident[:])
            epst = constp.tile([128, 1], F32)
            nc.vector.memset(epst[:], EPS)
            ones = constp.tile([128, 1], F32)
            nc.vector.memset(ones[:], 1.0)

            # ---- input DMAs
            praw = inp.tile([128, 96], F32)  # point n = 32p + m -> [p, m*3+j]
            nc.sync.dma_start(praw[:], xp_d[:].rearrange("(p m) j -> p (m j)", p=128))
            traw = inp.tile([128, 96], F32)
            nc.sync.dma_start(traw[:], xt_d[:].rearrange("(p m) j -> p (m j)", p=128))

            # frame coords: 3 tiles [128, 24] = (tensor2, chunk4, comp3)
            fc = []
            for s in range(3):
                t = inp.tile([128, 24], F32, tag=f"fc{s}")
                for t0, fd in ((0, fp_d), (1, ft_d)):
                    nc.sync.dma_start(
                        t[:, t0 * 12: t0 * 12 + 12].rearrange("p (c j) -> p c j", j=3),
                        fd[s: s + FPC].rearrange("(c p) j -> p c j", p=128),
                    )
                fc.append(t)
            vm_sb = inp.tile([128, 4], F32)
            nc.sync.dma_start(vm_sb[:], vm_d[:].rearrange("(c p) -> p c", p=128))

            # ---- X features: Xall[p, g*32 + k], g = 0..31 point-groups
            xall = xf.tile([128, XCOLS], F32)
            nc.vector.memset(xall[:], 0.0)
            xg = xall[:].rearrange("p (m k) -> p m k", k=KPAD)[:, 0:NGRP, :]

            pv = praw[:].rearrange("p (m j) -> p m j", j=3)
            tv = traw[:].rearrange("p (m j) -> p m j", j=3)

            # A = |p|^2 + |t|^2: square both, group-reduce, add
            sqp = xf.tile([128, 96], F32)
            nc.vector.tensor_mul(sqp[:], praw[:], praw[:])
            nc.vector.reduce_sum(
                xg[:, :, 0], sqp[:].rearrange("p (m j) -> p m j", j=3),
                axis=mybir.AxisListType.X,
            )
            nc.vector.tensor_mul(sqp[:], traw[:], traw[:])
            scr = xf.tile([128, 32], F32)
            nc.vector.reduce_sum(scr[:], sqp[:].rearrange("p (m j) -> p m j", j=3), axis=mybir.AxisListType.X)
            nc.vector.tensor_add(xg[:, :, 0], xg[:, :, 0], scr[:])
            nc.vector.memset(xg[:, :, 1], 1.0)
            # p, t coords (single strided copies)
            nc.vector.tensor_copy(xg[:, :, 2:5], pv)
            nc.vector.tensor_copy(xg[:, :, 5:8], tv)
            # W = outer(p, t): one op via double broadcast
            wout = xg[:, :, 8:17].rearrange("p m (c d) -> p m c d", d=3)
            pb = pv[:, :, :, None].broadcast_to([128, NGRP, 3, 3])
            tb = tv[:, :, None, :].broadcast_to([128, NGRP, 3, 3])
            nc.vector.tensor_mul(wout, pb, tb)

            # ---- transpose X in 96-col windows (3 groups per window so all
            # lhsT slices start at partition 0/32/64; 96 is illegal for PE)
            xts = []
            for b in range(NBLK):
                ps = psT.tile([96, 128], F32, tag="ps_tp")
                nc.tensor.transpose(ps[:], xall[:, b * 96: b * 96 + 96], ident[:])
                xt_t = xtb.tile([96, 128], MMDT, tag="xt_t")
                nc.scalar.copy(xt_t[:], ps[:])
                xts.append(xt_t)

            # ---- Y features (frames on partitions, pred+true batched)
            e1n, e2n, e3, orig = _frame_basis(nc, yp, fc)
            po = orig[:, 0:12]
            to = orig[:, 12:24]

            V3 = lambda ap: ap.rearrange("p (c j) -> p c j", j=3)
            V33 = lambda ap: ap.rearrange("p (c i j) -> p c i j", i=3, j=3)

            # M[c,d] = sum_r ep_r[c] et_r[d]  -> [128, 36] = (chunk4, c3, d3)
            mw = yp.tile([128, 36], F32)
            mt = yp.tile([128, 36], F32)
            for r, e in enumerate((e1n, e2n, e3)):
                epb = V3(e[:, 0:12])[:, :, :, None].broadcast_to([128, 4, 3, 3])
                etb = V3(e[:, 12:24])[:, :, None, :].broadcast_to([128, 4, 3, 3])
                if r == 0:
                    nc.vector.tensor_mul(V33(mw[:]), epb, etb)
                else:
                    nc.vector.tensor_mul(V33(mt[:]), epb, etb)
                    nc.vector.tensor_add(mw[:], mw[:], mt[:])

            # u = M to (reduce over d), v = M^T po (reduce over c)
            prod = yp.tile([128, 36], F32)
            tob = V3(to)[:, :, None, :].broadcast_to([128, 4, 3, 3])
            nc.vector.tensor_mul(V33(prod[:]), V33(mw[:]), tob)
            u = yp.tile([128, 12], F32)
            nc.vector.reduce_sum(V3(u[:]), V33(prod[:]), axis=mybir.AxisListType.X)

            mw_t = V33(mw[:]).transpose([0, 1, 3, 2])  # (chunk, d, c)
            pob = V3(po)[:, :, None, :].broadcast_to([128, 4, 3, 3])
            nc.vector.tensor_mul(V33(prod[:]), mw_t, pob)
            v = yp.tile([128, 12], F32)
            nc.vector.reduce_sum(V3(v[:]), V33(prod[:]), axis=mybir.AxisListType.X)

            # c_f = po . u ; B = ||po||^2 + ||to||^2
            sm = yp.tile([128, 24], F32)
            nc.vector.tensor_mul(sm[:, 0:12], po, u[:])
            cf = yp.tile([128, 4], F32)
            nc.vector.reduce_sum(cf[:], V3(sm[:, 0:12]), axis=mybir.AxisListType.X)
            nc.vector.tensor_mul(sm[:], orig[:], orig[:])
            b8 = yp.tile([128, 8], F32)
            nc.vector.reduce_sum(
                b8[:].rearrange("p (t c) -> p t c", t=2),
                sm[:].rearrange("p (t c j) -> p t c j", t=2, j=3),
                axis=mybir.AxisListType.X,
            )
            bsum = yp.tile([128, 4], F32)
            nc.vector.tensor_add(bsum[:], b8[:, 0:4], b8[:, 4:8])
            nc.vector.tensor_scalar_add(bsum[:], bsum[:], DSQ_OFF)

            # ---- assemble Yassem [128, 128] = (chunk4, feat32)
            yassem = yp.tile([128, FCHUNK * KPAD], F32)
            nc.vector.memset(yassem[:], 0.0)
            yv = yassem[:].rearrange("p (c k) -> p c k", k=KPAD)

            nc.vector.memset(yv[:, :, 0], 1.0)
            # k1 = B - 2 c_f
            nc.vector.scalar_tensor_tensor(
                yv[:, :, 1], cf[:], -2.0, bsum[:], OP.mult, OP.add
            )
            # k2..4 = 2(u - po) ; k5..7 = 2(v - to)
            diff = yp.tile([128, 12], F32)
            nc.vector.tensor_sub(diff[:], u[:], po)
            nc.vector.tensor_scalar_mul(yv[:, :, 2:5], V3(diff[:]), 2.0)
            nc.vector.tensor_sub(diff[:], v[:], to)
            nc.vector.tensor_scalar_mul(yv[:, :, 5:8], V3(diff[:]), 2.0)
            # k8..16 = -2 M
            nc.vector.tensor_scalar_mul(
                yv[:, :, 8:17], V33(mw[:]).rearrange("p c i j -> p c (i j)"), -2.0
            )
            # mask out pad frames (vmask = 0): multiply every feature
            vb = vm_sb[:][:, :, None].broadcast_to([128, 4, KPAD])
            nc.vector.tensor_mul(yv[:, :, 0:KPAD], yv[:, :, 0:KPAD], vb)

            # ---- replicate features 4x within each chunk's 128-col window,
            # transpose per chunk, evac into rhs4 [128, 512]: rows r*32+k all
            # hold Y^T so lhsT slices at bases 0/32/64 find a matching rhs
            yrep = yp.tile([128, FCHUNK * 128], F32)
            yrv = yrep[:].rearrange("p (c r k) -> p c r k", r=4, k=KPAD)
            ysrc = yv[:, :, None, :].broadcast_to([128, 4, 4, KPAD])
            nc.vector.tensor_copy(yrv, ysrc)

            rhs4 = yp.tile([128, FPC], MMDT)
            for c in range(FCHUNK):
                psy = psT.tile([128, 128], F32, tag="ps_tp")
                nc.tensor.transpose(psy[:], yrep[:, c * 128: (c + 1) * 128], ident[:])
                nc.scalar.copy(rhs4[:, c * 128: (c + 1) * 128], psy[:])

            # ---- main: 16 x (2 matmuls K=17 -> ACT sqrt PSUM->bf16 ->
            # DVE fused clamp+sum) on [128, 1024] double-bank PSUM tiles
            zerot = constp.tile([128, 2 * FPC], BF16)
            nc.vector.memset(zerot[:], 0.0)
            acc = accp.tile([128, NGRP // 2], F32)
            for i in range(NGRP // 2):
                ps = psD.tile([128, 2 * FPC], F32, tag="d2")
                for h in range(2):
                    g = 2 * i + h
                    b, s = divmod(g, 3)
                    lhsT = xts[b][s * KPAD: s * KPAD + KF, :]
                    rhs_r = rhs4[s * KPAD: s * KPAD + KF, :]
                    nc.tensor.matmul(
                        ps[:, h * FPC: (h + 1) * FPC],
                        lhsT, rhs_r,
                        start=True, stop=True,
                    )
                ssq = post.tile([128, 2 * FPC], BF16, tag="ssq")
                nc.scalar.activation(ssq[:], ps[:], AF.Sqrt, bias=epst[:])
                clp = post.tile([128, 2 * FPC], BF16, tag="clp")
                nc.vector.scalar_tensor_tensor(
                    clp[:], ssq[:], 10.0, zerot[:], OP.min, OP.max,
                    accum_out=acc[:, i: i + 1],
                )

            # ---- tail: acc [128,16] -> [128,1] -> [1,1]
            accs = accp.tile([128, 1], F32)
            nc.vector.reduce_sum(accs[:], acc[:], axis=mybir.AxisListType.X)
            psf = psT.tile([1, 1], F32, tag="ps_tp")
            nc.tensor.matmul(psf[:], accs[:], ones[:], start=True, stop=True)
            outsb = accp.tile([1, 1], F32)
            nc.vector.tensor_copy(outsb[:], psf[:])
            nc.sync.dma_start(out_d[:], outsb[:])

    nc.finalize()
    return nc


_NC_CACHE = None


def _get_nc():
    global _NC_CACHE
    if _NC_CACHE is None:
        _NC_CACHE = build_nc()
    return _NC_CACHE


def make_in_maps(pred_coords, true_coords):
    pred = np.ascontiguousarray(pred_coords, dtype=np.float32)
    true = np.ascontiguousarray(true_coords, dtype=np.float32)
    in_maps = []
    for i in range(NCORES):
        f0 = i * FPC
        fp = np.zeros((FPC + 2, 3), np.float32)
        ft = np.zeros((FPC + 2, 3), np.float32)
        hi = min(f0 + FPC + 2, N)
        fp[: hi - f0] = pred[f0:hi]
        ft[: hi - f0] = true[f0:hi]
        vm = np.ones(FPC, np.float32)
        nvalid = max(0, min(FPC, F - f0))
        vm[nvalid:] = 0.0
        in_maps.append({"xp": pred, "xt": true, "fp": fp, "ft": ft, "vm": vm})
    return in_maps


def kernel(pred_coords, true_coords):
    nc = _get_nc()
    in_maps = make_in_maps(pred_coords, true_coords)
    res = run_bass_kernel_spmd(nc, in_maps, list(range(NCORES)))
    total = sum(float(r["out"][0, 0]) for r in res.results)
    return np.float32(total / (F * N) / UNIT)
